# revision 17
# baseline (speedup 1.0000x reference)
"""Causal self-attention (B=2, T=2048, C=1024, H=16) on 8 Trainium2 cores.

Sharding: data-parallel over batch (2) x tensor-parallel over heads (4 groups
of 4 heads). Core c handles batch b = c//4, head group g = c%4 (heads 4g..4g+3).
Each core computes its qkv column slice, full causal TxT attention for its 4
heads, and a partial row-parallel projection. Host sums the 4 partial proj
outputs per batch and adds b_proj.

v3 (merged pipeline) design notes:
- single scheduling region: qkv GEMM, attention, and proj matmuls are
  interleaved in one PE stream so ACT-engine exp work (~80us, the attention
  critical path) hides under qkv/proj matmuls instead of serializing after
  them. All matmul operands are bf16 (PSUM accumulation stays f32): rel err
  ~4e-3 vs the 2e-2 gate, halves DMA bytes and SBUF, and enables PE fast
  weight loads.
- startup: the PE HAM clock gate defaults to half clock and takes ~3.4us of
  sustained work to lift; 34 warmup matmuls on a memset tile (no DMA
  dependency) run from ~0.5us while the first input slices land. DMA has a
  ~9us boot latency and ~330GB/s wire rate, so x arrives in 4 t-slice chunks
  (sync HWDGE) with weights in parallel on the gpsimd SWDGE queue.
- block pipeline: scores for chunk jc+1 issue before the AV matmuls of chunk
  jc, so the PE never sits directly behind the ACT exp latency; PSUM is
  exactly 8 banks: 2 rotating score tiles [P,2,512] (shared tag with proj /
  norm-broadcast / qkv groups) + 2 rotating AV-pair accumulators [P,2,512].
- filler units (qkv t-chunks for later i-blocks, v chunks, proj tiles for
  earlier i-blocks) are split into 2 closures (4+4 accumulation matmuls) and
  dealt into the attention blocks at unit-part granularity, early/late-half
  scheduled so lagged softmax-denominator norms pop exactly at mid-block.
- uniform causal diagonal: chunk o in 0..3 narrows score/exp/AV columns to
  [o*128, 512) plus one [128,128] triangular multiplicative mask.
- softmax denominators ride free in the AV matmul via an embedded all-ones
  column (v stationary zero-padded to M=128); one selector matmul broadcasts
  both heads' denominator rows across partitions; reciprocal_approx_fast
  (~5x cheaper than DVE reciprocal, ~18 good bits) finishes normalization.
"""

import os
import sys

sys.path.insert(0, "/opt/trn_rl_repo")

import numpy as np
import ml_dtypes

P = 128
T = 2048
C = 1024
D = 64
HPC = 4          # heads per core
HD = HPC * D     # 256 qkv columns per core
CC = C // P      # 8 contraction chunks
TC = T // P      # 16 t-chunks of 128
IC = T // 512    # 4 i-chunks of 512
N_WARM = 26

_NC = None
LAST_RESULTS = None


def _build_nc():
    import concourse.mybir as mybir
    import concourse.tile as tile
    from concourse import bacc
    from contextlib import ExitStack

    dt = mybir.dt
    f32 = dt.float32
    bf16 = dt.bfloat16
    ALU = mybir.AluOpType
    ACTF = mybir.ActivationFunctionType

    nc = bacc.Bacc(
        "TRN2",
        target_bir_lowering=False,
        debug=False,
        enable_asserts=False,
        num_devices=8,
    )

    xT = nc.dram_tensor("xT", [C, T], bf16, kind="ExternalInput").ap()
    wq = nc.dram_tensor("wq", [C, HD], bf16, kind="ExternalInput").ap()
    wk = nc.dram_tensor("wk", [C, HD], bf16, kind="ExternalInput").ap()
    wv = nc.dram_tensor("wv", [C, HD], bf16, kind="ExternalInput").ap()
    wp = nc.dram_tensor("wp", [HD, C], bf16, kind="ExternalInput").ap()
    # bf16 constant bundle: selector [P,128] | tri [P,128]
    cbd = nc.dram_tensor("cbd", [P, 256], bf16, kind="ExternalInput").ap()
    # f32 constant bundle: bq [P,2] | bk [P,2] | bv broadcast [P,256]
    cfd = nc.dram_tensor("cfd", [P, 260], f32, kind="ExternalInput").ap()
    out = nc.dram_tensor("out", [T, C], bf16, kind="ExternalOutput").ap()

    with tile.TileContext(nc) as tc, ExitStack() as ctx:
        persist = ctx.enter_context(tc.tile_pool(name="persist", bufs=1))
        qTb = persist.tile([P, 2, T], bf16, name="qTb")   # [d%128, pair, t]
        kTb = persist.tile([P, 2, T], bf16, name="kTb")
        vb = persist.tile([P, TC, 2, 2, P], bf16, name="vb")  # [t%128, tj, hp, hi, dpad]
        yTb = persist.tile([P, 2, T], bf16, name="yTb")
        wpb = persist.tile([P, 2, C], bf16, name="wpb")
        cb = persist.tile([P, 256], bf16, name="cb")
        cf = persist.tile([P, 260], f32, name="cf")
        dsb = persist.tile([P, 512], bf16, name="dsb")
        warm = persist.tile([P, 512], bf16, name="warm")
        xTs = persist.tile([P, CC, T], bf16, name="xTs")
        wqs = persist.tile([P, CC, HD], bf16, name="wqs")
        wks = persist.tile([P, CC, HD], bf16, name="wks")
        wvs = persist.tile([P, CC, HD], bf16, name="wvs")

        selb = cb[:, 0:128]
        trib = cb[:, 128:256]
        bvv = cf[:, 4:260].rearrange("p (hp hi d) -> p hp hi d", hi=2, d=D)

        exp_pool = ctx.enter_context(tc.tile_pool(name="exp", bufs=4))
        ot_pool = ctx.enter_context(tc.tile_pool(name="otp", bufs=3))
        rc_pool = ctx.enter_context(tc.tile_pool(name="rcp", bufs=2))
        ps_s = ctx.enter_context(tc.tile_pool(name="pss", bufs=2, space="PSUM"))
        ps_a = ctx.enter_context(tc.tile_pool(name="psa", bufs=2, space="PSUM"))

        # ---- HAM warmup: PE busy from ~0.5us (no DMA dependency) ----
        nc.vector.memset(warm[:], 0.0)
        wps = ps_s.tile([P, 2, 512], f32, tag="s")
        for _ in range(N_WARM):
            nc.tensor.matmul(
                wps[:, 0, :], warm[:, 0:P], warm[:, :],
                start=True, stop=True, skip_group_check=True,
            )

        # ---- input DMAs: the four x t-slices dispatch from four different
        # engines (vector/scalar are idle this early) so HWDGE descriptor
        # generation runs in parallel; consts + weights on the gpsimd SWDGE
        # path. x slice 0 + wk are the critical path for the first real MM.
        xTr = xT.rearrange("(o p) t -> p o t", p=P)
        for tsl, eng in ((0, nc.sync), (1, nc.scalar), (2, nc.sync), (3, nc.scalar)):
            eng.dma_start(
                xTs[:, :, tsl * 512:(tsl + 1) * 512], xTr[:, :, tsl * 512:(tsl + 1) * 512]
            )
        nc.gpsimd.dma_start(cb[:], cbd)
        nc.gpsimd.dma_start(cf[:], cfd)
        nc.gpsimd.dma_start(wks[:], wk.rearrange("(o p) n -> p o n", p=P))
        nc.gpsimd.dma_start(wqs[:], wq.rearrange("(o p) n -> p o n", p=P))
        nc.gpsimd.dma_start(wvs[:], wv.rearrange("(o p) n -> p o n", p=P))
        nc.gpsimd.dma_start(wpb[:], wp.rearrange("(o p) n -> p o n", p=P))
        # v padding: zeros everywhere, ones column at [hi=0, col D] / [hi=1,
        # col 0] (makes the AV matmul emit softmax denominator rows for free)
        nc.gpsimd.memset(vb[:], 0.0)
        nc.gpsimd.memset(vb[:, :, :, 0, D:D + 1], 1.0)
        nc.gpsimd.memset(vb[:, :, :, 1, 0:1], 1.0)
        # selector matmul contracts rows 1-63/65-127 of dsb with zero weights;
        # they must hold real zeros (not uninitialized bits)
        nc.gpsimd.memset(dsb[:], 0.0)

        # ---- emission helpers ----
        def qk_unit(dest, W_s, bcol, co, tsl):
            st = {}

            def mms(lo, hi):
                if lo == 0:
                    st["ps"] = ps_s.tile([P, 2, 512], f32, tag="s", name="fps")
                for cc in range(lo, hi):
                    nc.tensor.matmul(
                        st["ps"][:, 0, :],
                        W_s[:, cc, co * P:(co + 1) * P],
                        xTs[:, cc, tsl * 512:(tsl + 1) * 512],
                        start=(cc == 0),
                        stop=(cc == CC - 1),
                    )
                if hi == CC:
                    nc.vector.tensor_tensor(
                        dest[:, co, tsl * 512:(tsl + 1) * 512],
                        st["ps"][:, 0, :],
                        cf[:, bcol + co:bcol + co + 1].to_broadcast([P, 512]),
                        ALU.add,
                    )
            return [lambda: mms(0, 4), lambda: mms(4, CC)]

        def v_unit(tj):
            st = {}

            def mms(lo, hi):
                if lo == 0:
                    st["ps"] = ps_s.tile([P, 2, 512], f32, tag="s", name="fps")
                pv = st["ps"][:, 0, 0:HD]
                for cc in range(lo, hi):
                    nc.tensor.matmul(
                        pv,
                        xTs[:, cc, tj * P:(tj + 1) * P],
                        wvs[:, cc, :],
                        start=(cc == 0),
                        stop=(cc == CC - 1),
                    )
                if hi == CC:
                    psv = pv.rearrange("p (hp hi d) -> p hp hi d", hi=2, d=D)
                    nc.vector.tensor_tensor(
                        vb[:, tj, :, 0, 0:D], psv[:, :, 0, :], bvv[:, :, 0, :], ALU.add
                    )
                    nc.vector.tensor_tensor(
                        vb[:, tj, :, 1, D:P], psv[:, :, 1, :], bvv[:, :, 1, :], ALU.add
                    )
            return [lambda: mms(0, 4), lambda: mms(4, CC)]

        def proj_unit(ci, tjl):
            tj = 4 * ci + tjl
            st = {}

            def mms():
                st["ps"] = ps_s.tile([P, 2, 512], f32, tag="s", name="fps")
                for co in range(2):
                    for dc in range(2):
                        nc.tensor.matmul(
                            st["ps"][:, co, :],
                            yTb[:, dc, tj * P:(tj + 1) * P],
                            wpb[:, dc, co * 512:(co + 1) * 512],
                            start=(dc == 0),
                            stop=(dc == 1),
                        )

            def evac():
                ot = ot_pool.tile([P, C], bf16, tag="ot")
                nc.vector.tensor_copy(
                    ot[:].rearrange("p (a b) -> p a b", a=2), st["ps"][:, :, :]
                )
                nc.gpsimd.dma_start(out[tj * P:(tj + 1) * P, :], ot[:])
            return [mms, evac]

        def emit_norm(hp, ci, av):
            i0 = ci * 512
            nc.vector.tensor_copy(dsb[D:D + 1, :], av[D:D + 1, 0, :])  # den0 -> row 64
            nc.vector.tensor_copy(dsb[0:1, :], av[0:1, 1, :])          # den1 -> row 0
            bps = ps_s.tile([P, 2, 512], f32, tag="s")
            nc.tensor.matmul(
                bps[:, 0, :], selb, dsb[:], start=True, stop=True, skip_group_check=True
            )
            bsb = rc_pool.tile([P, 512], f32, tag="bsb")
            nc.vector.tensor_copy(bsb[:, :], bps[:, 0, :])
            rec = rc_pool.tile([P, 512], f32, tag="rec")
            nc.vector.reciprocal_approx_fast(rec[:, :], bsb[:, :])
            nc.vector.tensor_tensor(
                yTb[0:D, hp, i0:i0 + 512], av[0:D, 0, :], rec[0:D, :], ALU.mult
            )
            nc.vector.tensor_tensor(
                yTb[D:P, hp, i0:i0 + 512], av[D:P, 1, :], rec[D:P, :], ALU.mult
            )

        pending = []

        def emit_block(ci, hp, early, late):
            njc = 4 * (ci + 1)
            half = (njc + 1) // 2
            av = ps_a.tile([P, 2, 512], f32, tag="av")
            ei = li = 0
            prev = None
            for jc in range(njc):
                diag = jc >= 4 * ci
                o = jc - 4 * ci
                c0 = o * P if diag else 0
                sps = ps_s.tile([P, 2, 512], f32, tag="s")
                for hi in range(2):
                    bp = D * hi
                    nc.tensor.matmul(
                        sps[:, hi, c0:512],
                        kTb[bp:bp + D, hp, jc * P:(jc + 1) * P],
                        qTb[bp:bp + D, hp, ci * 512 + c0:ci * 512 + 512],
                        start=True,
                        stop=True,
                        skip_group_check=True,
                    )
                ex = exp_pool.tile([P, 2, 512], bf16, tag="ex")
                if jc == 0:
                    # split by head-half: av(0) can issue after the first half
                    for hi in range(2):
                        nc.scalar.activation(
                            ex[:, hi, c0:512], sps[:, hi, c0:512], ACTF.Exp,
                            scale=float(D) ** -0.5,
                        )
                else:
                    nc.scalar.activation(
                        ex[:, :, c0:512], sps[:, :, c0:512], ACTF.Exp,
                        scale=float(D) ** -0.5,
                    )
                if diag:
                    nc.vector.tensor_tensor(
                        ex[:, :, c0:c0 + P],
                        ex[:, :, c0:c0 + P],
                        trib[:, None, :].to_broadcast([P, 2, P]),
                        ALU.mult,
                    )
                # AV lags one chunk so the PE never sits behind the exp
                if prev is not None:
                    emit_av(av, njc, hp, *prev)
                prev = (jc, ex, c0)
                if jc + 1 == half and pending:
                    emit_norm(*pending.pop(0))
                if jc + 1 <= half:
                    want = (jc + 1) * len(early) // half
                    while ei < want:
                        early[ei]()
                        ei += 1
                else:
                    want = (jc + 1 - half) * len(late) // (njc - half)
                    while li < want:
                        late[li]()
                        li += 1
            emit_av(av, njc, hp, *prev)
            while ei < len(early):
                early[ei]()
                ei += 1
            while li < len(late):
                late[li]()
                li += 1
            pending.append((hp, ci, av))

        def emit_av(av, njc, hp, jc, ex, c0):
            for hi in range(2):
                nc.tensor.matmul(
                    av[:, hi, c0:512],
                    vb[:, jc, hp, hi, :],
                    ex[:, hi, c0:512],
                    start=(jc == 0),
                    stop=(jc == njc - 1),
                    skip_group_check=True,
                )

        # ---- prologue: everything block (0,*) needs ----
        for co in range(2):
            for f in qk_unit(kTb, wks, 2, co, 0):
                f()
        for tj in range(4):
            for f in v_unit(tj):
                f()
        for co in range(2):
            for f in qk_unit(qTb, wqs, 0, co, 0):
                f()

        # ---- main loop: attention blocks with interleaved filler parts ----
        def parts(*units):
            return [p for u in units for p in u]


        emit_block(0, 0, qk_unit(kTb, wks, 2, 0, 1), qk_unit(kTb, wks, 2, 1, 1))

        emit_block(0, 1, qk_unit(qTb, wqs, 0, 0, 1), qk_unit(qTb, wqs, 0, 1, 1))

        emit_block(1, 0, parts(v_unit(4), v_unit(5), v_unit(6), v_unit(7)),
                   parts(qk_unit(kTb, wks, 2, 0, 2), qk_unit(kTb, wks, 2, 1, 2)))

        emit_block(1, 1, parts(qk_unit(qTb, wqs, 0, 0, 2), qk_unit(qTb, wqs, 0, 1, 2)),
                   parts(proj_unit(0, 0), proj_unit(0, 1), proj_unit(0, 2), proj_unit(0, 3)))

        emit_block(2, 0, parts(v_unit(8), v_unit(9), v_unit(10), v_unit(11)),
                   parts(qk_unit(kTb, wks, 2, 0, 3), qk_unit(kTb, wks, 2, 1, 3)))

        emit_block(2, 1, parts(qk_unit(qTb, wqs, 0, 0, 3), qk_unit(qTb, wqs, 0, 1, 3)),
                   parts(proj_unit(1, 0), proj_unit(1, 1), proj_unit(1, 2), proj_unit(1, 3)))

        emit_block(3, 0, parts(v_unit(12), v_unit(13), v_unit(14), v_unit(15)),
                   parts(proj_unit(2, 0), proj_unit(2, 1)))

        emit_block(3, 1, parts(proj_unit(2, 2), proj_unit(2, 3)), [])
        while pending:
            emit_norm(*pending.pop(0))
        for tjl in range(4):
            for f in proj_unit(3, tjl):
                f()
    nc.compile()
    return nc


def _get_nc():
    global _NC
    if _NC is None:
        _NC = _build_nc()
    return _NC


def make_in_map(x_b, W_qkv, b_qkv, W_proj, g):
    """Per-core input map for batch row x_b [T, C] and head group g."""
    bf16 = ml_dtypes.bfloat16
    s = slice(HD * g, HD * g + HD)
    sk = slice(C + HD * g, C + HD * g + HD)
    sv = slice(2 * C + HD * g, 2 * C + HD * g + HD)
    cbd = np.zeros((P, 256), dtype=bf16)
    cbd[D, 0:D] = 1.0                                     # selector: den0 -> rows 0-63
    cbd[0, D:128] = 1.0                                   # selector: den1 -> rows 64-127
    cbd[:, 128:256] = np.triu(np.ones((P, P), np.float32)).astype(bf16)
    cfd = np.zeros((P, 260), dtype=np.float32)
    cfd[:, 0:2] = b_qkv[s].reshape(2, P).T
    cfd[:, 2:4] = b_qkv[sk].reshape(2, P).T
    cfd[:, 4:260] = np.broadcast_to(b_qkv[sv], (P, HD))
    return {
        "xT": np.ascontiguousarray(x_b.T).astype(bf16),
        "wq": np.ascontiguousarray(W_qkv[:, s]).astype(bf16),
        "wk": np.ascontiguousarray(W_qkv[:, sk]).astype(bf16),
        "wv": np.ascontiguousarray(W_qkv[:, sv]).astype(bf16),
        "wp": np.ascontiguousarray(W_proj[s, :]).astype(bf16),
        "cbd": cbd,
        "cfd": np.ascontiguousarray(cfd),
    }


def kernel(x, W_qkv, b_qkv, W_proj, b_proj):
    global LAST_RESULTS
    from concourse import bass_utils

    x = np.asarray(x, dtype=np.float32)
    W_qkv = np.asarray(W_qkv, dtype=np.float32)
    b_qkv = np.asarray(b_qkv, dtype=np.float32)
    W_proj = np.asarray(W_proj, dtype=np.float32)
    b_proj = np.asarray(b_proj, dtype=np.float32)

    nc = _get_nc()
    in_maps = []
    for c in range(8):
        b, g = divmod(c, 4)
        in_maps.append(make_in_map(x[b], W_qkv, b_qkv, W_proj, g))

    res = bass_utils.run_bass_kernel_spmd(nc, in_maps, core_ids=list(range(8)))
    LAST_RESULTS = res
    ys = []
    for b in range(2):
        y = np.asarray(res.results[4 * b]["out"], dtype=np.float64)
        for g in range(1, 4):
            y = y + np.asarray(res.results[4 * b + g]["out"], dtype=np.float64)
        ys.append((y + b_proj).astype(np.float32))
    return np.stack(ys, axis=0)


# revision 18
# speedup vs baseline: 1.0107x; 1.0107x over previous
"""Causal self-attention (B=2, T=2048, C=1024, H=16) on 8 Trainium2 cores.

Sharding: data-parallel over batch (2) x tensor-parallel over heads (4 groups
of 4 heads). Core c handles batch b = c//4, head group g = c%4 (heads 4g..4g+3).
Each core computes its qkv column slice, full causal TxT attention for its 4
heads, and a partial row-parallel projection. Host sums the 4 partial proj
outputs per batch and adds b_proj.

v3 (merged pipeline) design notes:
- single scheduling region: qkv GEMM, attention, and proj matmuls are
  interleaved in one PE stream so ACT-engine exp work (~80us, the attention
  critical path) hides under qkv/proj matmuls instead of serializing after
  them. All matmul operands are bf16 (PSUM accumulation stays f32): rel err
  ~4e-3 vs the 2e-2 gate, halves DMA bytes and SBUF, and enables PE fast
  weight loads.
- startup: the PE HAM clock gate defaults to half clock and takes ~3.4us of
  sustained work to lift; 34 warmup matmuls on a memset tile (no DMA
  dependency) run from ~0.5us while the first input slices land. DMA has a
  ~9us boot latency and ~330GB/s wire rate, so x arrives in 4 t-slice chunks
  (sync HWDGE) with weights in parallel on the gpsimd SWDGE queue.
- block pipeline: scores for chunk jc+1 issue before the AV matmuls of chunk
  jc, so the PE never sits directly behind the ACT exp latency; PSUM is
  exactly 8 banks: 2 rotating score tiles [P,2,512] (shared tag with proj /
  norm-broadcast / qkv groups) + 2 rotating AV-pair accumulators [P,2,512].
- filler units (qkv t-chunks for later i-blocks, v chunks, proj tiles for
  earlier i-blocks) are split into 2 closures (4+4 accumulation matmuls) and
  dealt into the attention blocks at unit-part granularity, early/late-half
  scheduled so lagged softmax-denominator norms pop exactly at mid-block.
- uniform causal diagonal: chunk o in 0..3 narrows score/exp/AV columns to
  [o*128, 512) plus one [128,128] triangular multiplicative mask.
- softmax denominators ride free in the AV matmul via an embedded all-ones
  column (v stationary zero-padded to M=128); one selector matmul broadcasts
  both heads' denominator rows across partitions; reciprocal_approx_fast
  (~5x cheaper than DVE reciprocal, ~18 good bits) finishes normalization.
"""

import os
import sys

sys.path.insert(0, "/opt/trn_rl_repo")

import numpy as np
import ml_dtypes

P = 128
T = 2048
C = 1024
D = 64
HPC = 4          # heads per core
HD = HPC * D     # 256 qkv columns per core
CC = C // P      # 8 contraction chunks
TC = T // P      # 16 t-chunks of 128
IC = T // 512    # 4 i-chunks of 512
N_WARM = 10

_NC = None
LAST_RESULTS = None


def _build_nc():
    import concourse.mybir as mybir
    import concourse.tile as tile
    from concourse import bacc
    from contextlib import ExitStack

    dt = mybir.dt
    f32 = dt.float32
    bf16 = dt.bfloat16
    ALU = mybir.AluOpType
    ACTF = mybir.ActivationFunctionType

    nc = bacc.Bacc(
        "TRN2",
        target_bir_lowering=False,
        debug=False,
        enable_asserts=False,
        num_devices=8,
    )

    xT = nc.dram_tensor("xT", [C, T], bf16, kind="ExternalInput").ap()
    wq = nc.dram_tensor("wq", [C, HD], bf16, kind="ExternalInput").ap()
    wk = nc.dram_tensor("wk", [C, HD], bf16, kind="ExternalInput").ap()
    wv = nc.dram_tensor("wv", [C, HD], bf16, kind="ExternalInput").ap()
    wp = nc.dram_tensor("wp", [HD, C], bf16, kind="ExternalInput").ap()
    # bf16 constant bundle: selector [P,128] | tri [P,128]
    cbd = nc.dram_tensor("cbd", [P, 256], bf16, kind="ExternalInput").ap()
    # f32 constant bundle: bq [P,2] | bk [P,2] | bv broadcast [P,256]
    cfd = nc.dram_tensor("cfd", [P, 260], f32, kind="ExternalInput").ap()
    out = nc.dram_tensor("out", [T, C], bf16, kind="ExternalOutput").ap()

    with tile.TileContext(nc) as tc, ExitStack() as ctx:
        persist = ctx.enter_context(tc.tile_pool(name="persist", bufs=1))
        qTb = persist.tile([P, 2, T], bf16, name="qTb")   # [d%128, pair, t]
        kTb = persist.tile([P, 2, T], bf16, name="kTb")
        vb = persist.tile([P, TC, 2, 2, P], bf16, name="vb")  # [t%128, tj, hp, hi, dpad]
        yTb = persist.tile([P, 2, T], bf16, name="yTb")
        wpb = persist.tile([P, 2, C], bf16, name="wpb")
        cb = persist.tile([P, 256], bf16, name="cb")
        cf = persist.tile([P, 260], f32, name="cf")
        dsb = persist.tile([P, 512], bf16, name="dsb")
        warm = persist.tile([P, 512], bf16, name="warm")
        xTs = persist.tile([P, CC, T], bf16, name="xTs")
        wqs = persist.tile([P, CC, HD], bf16, name="wqs")
        wks = persist.tile([P, CC, HD], bf16, name="wks")
        wvs = persist.tile([P, CC, HD], bf16, name="wvs")

        selb = cb[:, 0:128]
        trib = cb[:, 128:256]
        bvv = cf[:, 4:260].rearrange("p (hp hi d) -> p hp hi d", hi=2, d=D)

        exp_pool = ctx.enter_context(tc.tile_pool(name="exp", bufs=4))
        ot_pool = ctx.enter_context(tc.tile_pool(name="otp", bufs=3))
        rc_pool = ctx.enter_context(tc.tile_pool(name="rcp", bufs=2))
        ps_s = ctx.enter_context(tc.tile_pool(name="pss", bufs=2, space="PSUM"))
        ps_a = ctx.enter_context(tc.tile_pool(name="psa", bufs=2, space="PSUM"))

        # ---- HAM warmup: PE busy from ~0.5us (no DMA dependency) ----
        nc.vector.memset(warm[:], 0.0)
        wps = ps_s.tile([P, 2, 512], f32, tag="s")
        for _ in range(N_WARM):
            nc.tensor.matmul(
                wps[:, 0, :], warm[:, 0:P], warm[:, :],
                start=True, stop=True, skip_group_check=True,
            )

        # ---- input DMAs: the four x t-slices dispatch from four different
        # engines (vector/scalar are idle this early) so HWDGE descriptor
        # generation runs in parallel; consts + weights on the gpsimd SWDGE
        # path. x slice 0 + wk are the critical path for the first real MM.
        xTr = xT.rearrange("(o p) t -> p o t", p=P)
        nc.sync.dma_start(xTs[:, 0:4, 0:512], xTr[:, 0:4, 0:512])
        nc.scalar.dma_start(xTs[:, 4:CC, 0:512], xTr[:, 4:CC, 0:512])
        for tsl, eng in ((1, nc.scalar), (2, nc.sync), (3, nc.scalar)):
            eng.dma_start(
                xTs[:, :, tsl * 512:(tsl + 1) * 512], xTr[:, :, tsl * 512:(tsl + 1) * 512]
            )
        nc.gpsimd.dma_start(cb[:], cbd)
        nc.gpsimd.dma_start(cf[:], cfd)
        nc.gpsimd.dma_start(wks[:], wk.rearrange("(o p) n -> p o n", p=P))
        nc.gpsimd.dma_start(wqs[:], wq.rearrange("(o p) n -> p o n", p=P))
        nc.gpsimd.dma_start(wvs[:], wv.rearrange("(o p) n -> p o n", p=P))
        nc.gpsimd.dma_start(wpb[:], wp.rearrange("(o p) n -> p o n", p=P))
        # v padding: zeros everywhere, ones column at [hi=0, col D] / [hi=1,
        # col 0] (makes the AV matmul emit softmax denominator rows for free)
        nc.gpsimd.memset(vb[:], 0.0)
        nc.gpsimd.memset(vb[:, :, :, 0, D:D + 1], 1.0)
        nc.gpsimd.memset(vb[:, :, :, 1, 0:1], 1.0)
        # selector matmul contracts rows 1-63/65-127 of dsb with zero weights;
        # they must hold real zeros (not uninitialized bits)
        nc.gpsimd.memset(dsb[:], 0.0)

        # ---- emission helpers ----
        def qk_unit(dest, W_s, bcol, co, tsl):
            st = {}

            def mms(lo, hi):
                if lo == 0:
                    st["ps"] = ps_s.tile([P, 2, 512], f32, tag="s", name="fps")
                for cc in range(lo, hi):
                    nc.tensor.matmul(
                        st["ps"][:, 0, :],
                        W_s[:, cc, co * P:(co + 1) * P],
                        xTs[:, cc, tsl * 512:(tsl + 1) * 512],
                        start=(cc == 0),
                        stop=(cc == CC - 1),
                    )
                if hi == CC:
                    nc.vector.tensor_tensor(
                        dest[:, co, tsl * 512:(tsl + 1) * 512],
                        st["ps"][:, 0, :],
                        cf[:, bcol + co:bcol + co + 1].to_broadcast([P, 512]),
                        ALU.add,
                    )
            return [lambda: mms(0, 4), lambda: mms(4, CC)]

        def v_unit(tj):
            st = {}

            def mms(lo, hi):
                if lo == 0:
                    st["ps"] = ps_s.tile([P, 2, 512], f32, tag="s", name="fps")
                pv = st["ps"][:, 0, 0:HD]
                for cc in range(lo, hi):
                    nc.tensor.matmul(
                        pv,
                        xTs[:, cc, tj * P:(tj + 1) * P],
                        wvs[:, cc, :],
                        start=(cc == 0),
                        stop=(cc == CC - 1),
                    )
                if hi == CC:
                    psv = pv.rearrange("p (hp hi d) -> p hp hi d", hi=2, d=D)
                    nc.vector.tensor_tensor(
                        vb[:, tj, :, 0, 0:D], psv[:, :, 0, :], bvv[:, :, 0, :], ALU.add
                    )
                    nc.vector.tensor_tensor(
                        vb[:, tj, :, 1, D:P], psv[:, :, 1, :], bvv[:, :, 1, :], ALU.add
                    )
            return [lambda: mms(0, 4), lambda: mms(4, CC)]

        def proj_unit(ci, tjl):
            tj = 4 * ci + tjl
            st = {}

            def mms():
                st["ps"] = ps_s.tile([P, 2, 512], f32, tag="s", name="fps")
                for co in range(2):
                    for dc in range(2):
                        nc.tensor.matmul(
                            st["ps"][:, co, :],
                            yTb[:, dc, tj * P:(tj + 1) * P],
                            wpb[:, dc, co * 512:(co + 1) * 512],
                            start=(dc == 0),
                            stop=(dc == 1),
                        )

            def evac():
                ot = ot_pool.tile([P, C], bf16, tag="ot")
                nc.vector.tensor_copy(
                    ot[:].rearrange("p (a b) -> p a b", a=2), st["ps"][:, :, :]
                )
                nc.gpsimd.dma_start(out[tj * P:(tj + 1) * P, :], ot[:])
            return [mms, evac]

        def emit_norm(hp, ci, av):
            i0 = ci * 512
            nc.vector.tensor_copy(dsb[D:D + 1, :], av[D:D + 1, 0, :])  # den0 -> row 64
            nc.vector.tensor_copy(dsb[0:1, :], av[0:1, 1, :])          # den1 -> row 0
            bps = ps_s.tile([P, 2, 512], f32, tag="s")
            nc.tensor.matmul(
                bps[:, 0, :], selb, dsb[:], start=True, stop=True, skip_group_check=True
            )
            bsb = rc_pool.tile([P, 512], f32, tag="bsb")
            nc.vector.tensor_copy(bsb[:, :], bps[:, 0, :])
            rec = rc_pool.tile([P, 512], f32, tag="rec")
            nc.vector.reciprocal_approx_fast(rec[:, :], bsb[:, :])
            nc.vector.tensor_tensor(
                yTb[0:D, hp, i0:i0 + 512], av[0:D, 0, :], rec[0:D, :], ALU.mult
            )
            nc.vector.tensor_tensor(
                yTb[D:P, hp, i0:i0 + 512], av[D:P, 1, :], rec[D:P, :], ALU.mult
            )

        pending = []

        def emit_block(ci, hp, early, late):
            njc = 4 * (ci + 1)
            half = (njc + 1) // 2
            av = ps_a.tile([P, 2, 512], f32, tag="av")
            ei = li = 0
            prev = None
            for jc in range(njc):
                diag = jc >= 4 * ci
                o = jc - 4 * ci
                c0 = o * P if diag else 0
                sps = ps_s.tile([P, 2, 512], f32, tag="s")
                for hi in range(2):
                    bp = D * hi
                    nc.tensor.matmul(
                        sps[:, hi, c0:512],
                        kTb[bp:bp + D, hp, jc * P:(jc + 1) * P],
                        qTb[bp:bp + D, hp, ci * 512 + c0:ci * 512 + 512],
                        start=True,
                        stop=True,
                        skip_group_check=True,
                    )
                ex = exp_pool.tile([P, 2, 512], bf16, tag="ex")
                nc.scalar.activation(
                    ex[:, :, c0:512], sps[:, :, c0:512], ACTF.Exp,
                    scale=float(D) ** -0.5,
                )
                if diag:
                    nc.vector.tensor_tensor(
                        ex[:, :, c0:c0 + P],
                        ex[:, :, c0:c0 + P],
                        trib[:, None, :].to_broadcast([P, 2, P]),
                        ALU.mult,
                    )
                # AV lags one chunk so the PE never sits behind the exp
                if prev is not None:
                    emit_av(av, njc, hp, *prev)
                prev = (jc, ex, c0)
                if jc + 1 == half and pending:
                    emit_norm(*pending.pop(0))
                if jc + 1 <= half:
                    want = (jc + 1) * len(early) // half
                    while ei < want:
                        early[ei]()
                        ei += 1
                else:
                    want = (jc + 1 - half) * len(late) // (njc - half)
                    while li < want:
                        late[li]()
                        li += 1
            emit_av(av, njc, hp, *prev)
            while ei < len(early):
                early[ei]()
                ei += 1
            while li < len(late):
                late[li]()
                li += 1
            pending.append((hp, ci, av))

        def emit_av(av, njc, hp, jc, ex, c0):
            for hi in range(2):
                nc.tensor.matmul(
                    av[:, hi, c0:512],
                    vb[:, jc, hp, hi, :],
                    ex[:, hi, c0:512],
                    start=(jc == 0),
                    stop=(jc == njc - 1),
                    skip_group_check=True,
                )

        # ---- prologue: everything block (0,*) needs ----
        for co in range(2):
            for f in qk_unit(kTb, wks, 2, co, 0):
                f()
        for tj in range(4):
            for f in v_unit(tj):
                f()
        for co in range(2):
            for f in qk_unit(qTb, wqs, 0, co, 0):
                f()

        # ---- main loop: attention blocks with interleaved filler parts ----
        def parts(*units):
            return [p for u in units for p in u]


        emit_block(0, 0, qk_unit(kTb, wks, 2, 0, 1), qk_unit(kTb, wks, 2, 1, 1))

        emit_block(0, 1, qk_unit(qTb, wqs, 0, 0, 1), qk_unit(qTb, wqs, 0, 1, 1))

        emit_block(1, 0, parts(v_unit(4), v_unit(5), v_unit(6), v_unit(7)),
                   parts(qk_unit(kTb, wks, 2, 0, 2), qk_unit(kTb, wks, 2, 1, 2)))

        emit_block(1, 1, parts(qk_unit(qTb, wqs, 0, 0, 2), qk_unit(qTb, wqs, 0, 1, 2)),
                   parts(proj_unit(0, 0), proj_unit(0, 1), proj_unit(0, 2), proj_unit(0, 3)))

        emit_block(2, 0, parts(v_unit(8), v_unit(9), v_unit(10), v_unit(11)),
                   parts(qk_unit(kTb, wks, 2, 0, 3), qk_unit(kTb, wks, 2, 1, 3)))

        emit_block(2, 1, parts(qk_unit(qTb, wqs, 0, 0, 3), qk_unit(qTb, wqs, 0, 1, 3)),
                   parts(proj_unit(1, 0), proj_unit(1, 1), proj_unit(1, 2), proj_unit(1, 3)))

        emit_block(3, 0, parts(v_unit(12), v_unit(13), v_unit(14), v_unit(15)),
                   parts(proj_unit(2, 0), proj_unit(2, 1)))

        emit_block(3, 1, parts(proj_unit(2, 2), proj_unit(2, 3)), [])
        while pending:
            emit_norm(*pending.pop(0))
        for tjl in range(4):
            for f in proj_unit(3, tjl):
                f()
    nc.compile()
    return nc


def _get_nc():
    global _NC
    if _NC is None:
        _NC = _build_nc()
    return _NC


def make_in_map(x_b, W_qkv, b_qkv, W_proj, g):
    """Per-core input map for batch row x_b [T, C] and head group g."""
    bf16 = ml_dtypes.bfloat16
    s = slice(HD * g, HD * g + HD)
    sk = slice(C + HD * g, C + HD * g + HD)
    sv = slice(2 * C + HD * g, 2 * C + HD * g + HD)
    cbd = np.zeros((P, 256), dtype=bf16)
    cbd[D, 0:D] = 1.0                                     # selector: den0 -> rows 0-63
    cbd[0, D:128] = 1.0                                   # selector: den1 -> rows 64-127
    cbd[:, 128:256] = np.triu(np.ones((P, P), np.float32)).astype(bf16)
    cfd = np.zeros((P, 260), dtype=np.float32)
    cfd[:, 0:2] = b_qkv[s].reshape(2, P).T
    cfd[:, 2:4] = b_qkv[sk].reshape(2, P).T
    cfd[:, 4:260] = np.broadcast_to(b_qkv[sv], (P, HD))
    return {
        "xT": np.ascontiguousarray(x_b.T).astype(bf16),
        "wq": np.ascontiguousarray(W_qkv[:, s]).astype(bf16),
        "wk": np.ascontiguousarray(W_qkv[:, sk]).astype(bf16),
        "wv": np.ascontiguousarray(W_qkv[:, sv]).astype(bf16),
        "wp": np.ascontiguousarray(W_proj[s, :]).astype(bf16),
        "cbd": cbd,
        "cfd": np.ascontiguousarray(cfd),
    }


def kernel(x, W_qkv, b_qkv, W_proj, b_proj):
    global LAST_RESULTS
    from concourse import bass_utils

    x = np.asarray(x, dtype=np.float32)
    W_qkv = np.asarray(W_qkv, dtype=np.float32)
    b_qkv = np.asarray(b_qkv, dtype=np.float32)
    W_proj = np.asarray(W_proj, dtype=np.float32)
    b_proj = np.asarray(b_proj, dtype=np.float32)

    nc = _get_nc()
    in_maps = []
    for c in range(8):
        b, g = divmod(c, 4)
        in_maps.append(make_in_map(x[b], W_qkv, b_qkv, W_proj, g))

    res = bass_utils.run_bass_kernel_spmd(nc, in_maps, core_ids=list(range(8)))
    LAST_RESULTS = res
    ys = []
    for b in range(2):
        y = np.asarray(res.results[4 * b]["out"], dtype=np.float64)
        for g in range(1, 4):
            y = y + np.asarray(res.results[4 * b + g]["out"], dtype=np.float64)
        ys.append((y + b_proj).astype(np.float32))
    return np.stack(ys, axis=0)
